# revision 28
# baseline (speedup 1.0000x reference)
"""GNN message passing (gather + scatter-add) on 8 trn2 NeuronCores, v4.

Strategy (dst-sharded, gather via InstDMAGatherAnt on 4 SWDGE queues):
  * The host ASSIGNS dst nodes to (core, tile, partition) slots with a
    load balancer (serpentine deal on degree + per-(tile,chunk) repair
    swaps) so every (group, chunk, tile) bin has ~equal edge count.
    This shrinks the SPMD-uniform bin capacities to ~mean (the baseline
    paid max-over-784-Poisson-bins), cutting msg/sel SBUF and blocks.
  * 112 tiles of 128 dst slots per core (TPG=16, 7 groups; 4 PSUM banks
    per group, 2 groups in flight = 8 banks). x stays in HBM as bf16;
    edges are binned by (group, chunk=25000 src rows, tile); each bin is
    gathered by ONE dma_gather (chunk-relative int16 idx), bins for
    adjacent tile pairs are merged into one gather (<=1024 ring descs)
    with mid-stream idx-0 pads (dst code 255 => sel column 0) and
    trailing -1 (trimmed by ucode via the per-core count register).
  * Descriptors can be 512B (DESC_ROWS=2: each desc fetches rows
    [src, src+2), the matmul reads cols 0:128) - measured ~16% faster
    per descriptor than 256B on the SWDGE queue pipeline.
  * One DVE is_equal per (group, chunk) step builds one-hot sel planes;
    one matmul per 128-slot block accumulates psum[dst,f] += sel^T@msg.
    Stale slots (count < capacity) keep old finite bf16 data and get
    sel 0. PSUM start/stop once per bank per group phase.
  * psum -> SBUF via scalar engine (whole-bank copies), HWDGE DMA out;
    host un-permutes rows via the balancer's node map.
No collective needed; each core owns its output rows.
"""

import os
import sys

import numpy as np
import ml_dtypes

for _p in ("/opt/trn_rl_repo",):
    if _p not in sys.path:
        sys.path.insert(0, _p)

import bass_rust  # noqa: E402
from concourse import bass, mybir, tile, bacc, library_config  # noqa: E402
from concourse.bass_utils import run_bass_kernel_spmd  # noqa: E402

P = 128
D = 128
N_NODES = 100000
N_CORES = 8

NBUF_S = int(os.environ.get("KERNEL_NBUF_S", "3"))  # sel buffers


def make_chunks(n_src, chunk):
    spans = []
    b = 0
    while b < n_src:
        s = min(chunk, n_src - b)
        spans.append((b, s))
        b += s
    return spans


def balance_nodes(src, dst, n_cores, tiles, spans):
    """Assign each dst node a (core, tile, partition) slot, balancing the
    per-(tile, chunk) edge counts. Returns node_slot [N] int64 encoding
    core*tiles*128 + tile*128 + p, with every tile holding <=128 nodes."""
    n = N_NODES
    nch = len(spans)
    ntile = n_cores * tiles
    # per-node per-chunk degree
    cid = np.minimum(src // spans[0][1], nch - 1)
    degc = np.zeros((nch, n), np.int32)
    for c in range(nch):
        degc[c] = np.bincount(dst[cid == c], minlength=n)
    deg = degc.sum(axis=0)

    # serpentine deal on total degree: round r gives one node to each tile,
    # pairing heaviest remaining nodes with lightest tiles.
    order = np.argsort(-deg, kind="stable")
    loads = np.zeros(ntile, np.int64)
    fill = np.zeros(ntile, np.int32)
    assign = np.empty(n, np.int64)
    pos = 0
    while pos < n:
        batch = order[pos : pos + ntile]
        tl = np.argsort(loads, kind="stable")[: len(batch)]
        assign[batch] = tl
        loads[tl] += deg[batch]
        fill[tl] += 1
        pos += len(batch)

    # repair pass: per-(tile, chunk) loads; swap high-deg nodes out of
    # overloaded bins into the lightest tiles (matched by total degree).
    cl = np.zeros((ntile, nch), np.int64)
    for c in range(nch):
        np.add.at(cl[:, c], assign, degc[c])
    lim = int(cl.mean()) + 16
    for _ in range(3000):
        worst = np.unravel_index(np.argmax(cl), cl.shape)
        t0, c0 = int(worst[0]), int(worst[1])
        if cl[t0, c0] <= lim:
            break
        cand = np.flatnonzero(assign == t0)
        mover = cand[np.argmax(degc[c0, cand])]
        t1 = int(np.argmin(cl[:, c0] + (fill >= 128) * (1 << 40)))
        # swap mover with a node in t1 of similar total degree but low c0 deg
        cand1 = np.flatnonzero(assign == t1)
        recv = cand1[np.argmin(degc[c0, cand1].astype(np.int64) * (1 << 20) - deg[cand1])]
        if degc[c0, mover] <= degc[c0, recv]:
            break
        assign[mover], assign[recv] = t1, t0
        cl[t0] += degc[:, recv] - degc[:, mover]
        cl[t1] += degc[:, mover] - degc[:, recv]

    # partition index within tile
    order2 = np.argsort(assign, kind="stable")
    idx_in_tile = np.empty(n, np.int64)
    start = 0
    counts = np.bincount(assign, minlength=ntile)
    assert counts.max() <= 128
    off = np.concatenate([[0], np.cumsum(counts)])
    ranks = np.arange(n) - off[assign[order2]]
    idx_in_tile[order2] = ranks
    node_slot = assign * P + idx_in_tile
    return node_slot  # global slot id: (core*tiles + tile)*128 + p


def merge_plan(blocks_step, max_blocks=8):
    """Greedy merge of adjacent tile bins into gathers of <= max_blocks*128
    descriptors (the SWDGE ring cap). Returns [(t_start, t_end_incl), ...]."""
    plan = []
    t = 0
    tpg = len(blocks_step)
    while t < tpg:
        e = t
        acc = int(blocks_step[t])
        while e + 1 < tpg and acc + int(blocks_step[e + 1]) <= max_blocks:
            e += 1
            acc += int(blocks_step[e])
        plan.append((t, e))
        t = e + 1
    return plan


def build_program(spans, caps, n_groups, tpg, num_devices, desc_rows, nbuf_m, nq):
    """caps: int array [n_groups, nch, tpg] = blocks per bin (uniform across
    cores). Gathers merge adjacent tile bins (<=1024 ring descriptors each).
    Output rows: n_groups*tpg*128."""
    nch = len(spans)
    E = D * desc_rows  # gathered elems per slot
    blocks = np.asarray(caps)  # [g][c][t]
    nblk = int(blocks.sum())
    step_blocks = blocks.sum(axis=2)  # [g][c]
    max_nb = int(step_blocks.max())
    nbins = n_groups * nch * tpg

    nc = bacc.Bacc(
        "TRN2",
        target_bir_lowering=False,
        debug=False,
        num_devices=num_devices,
        num_swdge_queues=nq,
        # ring of 2048 descriptors per SWDGE queue: two 1024-desc gathers in
        # flight per queue (deeper gen/drain pipelining, measured ~15% faster
        # per descriptor than the default 1024-desc ring).
        dynamic_dma_scratch_size=32768,
    )
    n_src = spans[-1][0] + spans[-1][1]
    xbf = nc.dram_tensor(
        "xbf", [n_src + 2 * desc_rows, D], mybir.dt.bfloat16, kind="ExternalInput"
    ).ap()
    ncol = nblk * P // 16
    idxT = nc.dram_tensor("idxT", [P, ncol], mybir.dt.int16, kind="ExternalInput").ap()
    dstT = nc.dram_tensor(
        "dstT", [P, nblk], mybir.dt.bfloat16, kind="ExternalInput"
    ).ap()
    iota = nc.dram_tensor(
        "iota", [P, P], mybir.dt.bfloat16, kind="ExternalInput"
    ).ap()
    out = nc.dram_tensor(
        "out", [n_groups * tpg * P, D], mybir.dt.float32, kind="ExternalOutput"
    ).ap()

    # slot offset (in blocks) of each bin, ordered (g, c, t)
    boff = np.zeros(nbins + 1, np.int64)
    boff[1:] = np.cumsum(blocks.reshape(-1))

    def bin_id(g, c, t):
        return (g * nch + c) * tpg + t

    with tile.TileContext(nc) as tc:
        with tc.tile_pool(name="sb", bufs=1) as pool, tc.tile_pool(
            name="ps", bufs=1, space="PSUM"
        ) as psp:
            idxs = pool.tile([P, ncol], mybir.dt.int16)
            dsts = pool.tile([P, nblk], mybir.dt.bfloat16)
            iot = pool.tile([P, P], mybir.dt.bfloat16)
            # per-step idx slices: the first gathers only wait on their own
            # slice, not the whole 4 MB index upload
            for g in range(n_groups):
                for c in range(nch):
                    s0 = int(boff[bin_id(g, c, 0)]) * P // 16
                    s1 = (
                        int(boff[bin_id(g, c, tpg - 1)] + blocks[g, c, tpg - 1])
                        * P
                        // 16
                    )
                    nc.sync.dma_start(out=idxs[:, s0:s1], in_=idxT[:, s0:s1])
            nc.sync.dma_start(out=dsts[:], in_=dstT[:])
            nc.sync.dma_start(out=iot[:], in_=iota[:])
            nc.gpsimd.load_library(library_config.mlp)

            msg = [
                pool.tile([P, max_nb, E], mybir.dt.bfloat16, name=f"msg{i}")
                for i in range(nbuf_m)
            ]
            sel = [
                pool.tile([P, max_nb, P], mybir.dt.bfloat16, name=f"sel{i}")
                for i in range(NBUF_S)
            ]
            stg = [
                pool.tile([P, tpg * D], mybir.dt.float32, name=f"stg{i}")
                for i in range(2)
            ]
            # no msg memsets: the first nbuf_m steps gather at FULL capacity
            # (host pads with idx 0 / dst 255), so stale slots always hold
            # finite bf16 data from a real row thereafter.
            bpg = -(-tpg // 4)  # banks per group
            assert 2 * bpg <= 8
            banks = [
                psp.tile([P, 4 * D], dtype=mybir.dt.float32, space="PSUM", name=f"psb{j}")
                for j in range(2 * bpg)
            ]

            def pregion(g, t):
                bk = banks[(g % 2) * bpg + t // 4]
                return bk[:, (t % 4) * D : (t % 4 + 1) * D]

            # one register per distinct gather capacity, set once: the ucode's
            # trailing -1 trim recovers each core's actual count, so no
            # per-gather reg_load is needed.
            plans = {
                (g, c): merge_plan(blocks[g, c])
                for g in range(n_groups)
                for c in range(nch)
            }
            capregs = {}
            for (g, c), plan in plans.items():
                for t0, t1 in plan:
                    ns = int(
                        boff[bin_id(g, c, t1)]
                        + blocks[g, c, t1]
                        - boff[bin_id(g, c, t0)]
                    ) * P
                    if ns not in capregs:
                        capregs[ns] = nc.gpsimd.alloc_register(f"cap{ns}")
            for ns, r in capregs.items():
                nc.gpsimd.reg_mov(r, ns)

            step = 0
            gq = 0
            for g in range(n_groups):
                for c in range(nch):
                    km = step % nbuf_m
                    ks = step % NBUF_S
                    mg, sl = msg[km], sel[ks]
                    base, span = spans[c]
                    nb = int(step_blocks[g, c])
                    sb0 = boff[bin_id(g, c, 0)]  # first block of this step
                    inap = xbf[base : base + span + 2 * desc_rows, :]
                    if desc_rows > 1:
                        # overlapping window view: row i -> elems [i*D, i*D+E)
                        inap = inap.copy()
                        inap.ap = bass_rust.VecI64Pair(
                            [(D, span + desc_rows), (1, E)]
                        )
                    for t0, t1 in plans[(g, c)]:
                        b0 = boff[bin_id(g, c, t0)]
                        bend = boff[bin_id(g, c, t1)] + blocks[g, c, t1]
                        nslot = int(bend - b0) * P
                        coloff = int(b0) * P // 16
                        nc.gpsimd.dma_gather(
                            mg[:, int(b0 - sb0) : int(bend - sb0), :],
                            inap,
                            idxs[:, coloff : coloff + nslot // 16],
                            nslot,
                            capregs[nslot],
                            E,
                            elem_step=D,
                            queue_num=gq % nq,
                            single_packet=bool(
                                int(os.environ.get("KERNEL_SP", "0"))
                            ),
                        )
                        gq += 1
                    nc.vector.tensor_tensor(
                        out=sl[:, :nb, :],
                        in0=dsts[:, int(sb0) : int(sb0 + nb)][:, :, None].to_broadcast(
                            [P, nb, P]
                        ),
                        in1=iot[:, None, :].to_broadcast([P, nb, P]),
                        op=mybir.AluOpType.is_equal,
                    )
                    # bank-interleaved tile order, block-outer: consecutive
                    # matmuls never hit the same psum region/bank.
                    torder = [t for r in range(4) for t in range(r, tpg, 4)]
                    last_of_bank = {}
                    maxb = int(blocks[g, c].max())
                    for b in range(maxb):
                        for t in torder:
                            if b < blocks[g, c, t]:
                                last_of_bank[t // 4] = (t, b)
                    started = set()
                    for b in range(maxb):
                        for t in torder:
                            if b >= blocks[g, c, t]:
                                continue
                            j = int(boff[bin_id(g, c, t)] - sb0) + b
                            bank = t // 4
                            start = c == 0 and b == 0 and bank not in started
                            if start:
                                started.add(bank)
                            nc.tensor.matmul(
                                out=pregion(g, t),
                                lhsT=sl[:, j, :],
                                rhs=mg[:, j, 0:D],
                                start=start,
                                stop=(
                                    c == nch - 1
                                    and last_of_bank[bank] == (t, b)
                                ),
                            )
                    step += 1
                sg = stg[g % 2]
                for k in range(bpg):
                    w = min(4, tpg - 4 * k)
                    bk = banks[(g % 2) * bpg + k]
                    nc.scalar.copy(
                        sg[:, 4 * k * D : (4 * k + w) * D], bk[:, : w * D]
                    )
                for t in range(tpg):
                    r0 = (g * tpg + t) * P
                    nc.sync.dma_start(
                        out=out[r0 : r0 + P, :], in_=sg[:, t * D : (t + 1) * D]
                    )

    for blk in nc.main_func.blocks:
        for ins in blk.instructions:
            if isinstance(ins, mybir.InstDMAGatherAnt):
                si = ins.sync_info
                if si and si.on_update:
                    name = si.on_update[0].ant_name
                    lane = int(name.split("_")[0][len("DMASW") :])
                    ins.queue_num = lane % nq
    nc.compile()
    return nc


def prep_core(src, rel, spans, caps, n_groups, tpg, nbuf_m=3):
    """Bin one core's edges (src global, rel = tile*128+p core-relative slot)
    into the (group, chunk, tile) layout. Returns (idxT, dstT).

    Gathers pass the (compile-time) capacity register; per-core counts are
    recovered by the ucode's trailing -1 trim. Mid-pads (first bin of each
    merged pair, and ALL pads in the first nbuf_m steps so msg buffers get
    fully initialized without memsets) are idx 0 with dst code 255."""
    nch = len(spans)
    blocks = np.asarray(caps)
    nblk = int(blocks.sum())
    nbins = n_groups * nch * tpg
    boff = np.zeros(nbins + 1, np.int64)
    boff[1:] = np.cumsum(blocks.reshape(-1))

    t = rel >> 7
    g = t // tpg
    ti = t % tpg
    chunk = spans[0][1]
    c = np.minimum(src // chunk, nch - 1)
    bases = np.array([b for b, s in spans], dtype=np.int64)
    sr = src - bases[c]

    binkey = (g * nch + c) * tpg + ti
    order = np.lexsort((sr, binkey))
    sr, relo, binkey = sr[order], rel[order], binkey[order]
    counts = np.bincount(binkey, minlength=nbins)
    capacity = blocks.reshape(-1) * P
    if (counts > capacity).any():
        raise ValueError("caps too small")
    starts = np.zeros(nbins, np.int64)
    starts[1:] = np.cumsum(counts)[:-1]
    pos = np.arange(len(sr)) - starts[binkey]
    slot = boff[binkey] * P + pos

    total = nblk * P
    srcflat = np.full(total, -1, np.int64)
    dstflat = np.full(total, 255, np.int64)  # 255 = stale/pad (sel col 0)
    srcflat[slot] = sr
    dstflat[slot] = relo & 127

    # pad everything except each merged gather's trailing bin with idx 0
    pad0 = np.zeros(nbins, bool)
    stepi = 0
    for gg in range(n_groups):
        for cc in range(nch):
            plan = merge_plan(blocks[gg, cc])
            for t0, t1 in plan:
                for tt in range(t0, t1 + 1):
                    k = (gg * nch + cc) * tpg + tt
                    pad0[k] = (tt < t1) or (stepi < nbuf_m)
            stepi += 1
    binf = np.repeat(np.arange(nbins), blocks.reshape(-1) * P)
    fill = (srcflat < 0) & pad0[binf]
    srcflat[fill] = 0

    idxT = np.empty((16, total // 16), np.int16)
    seg = srcflat.reshape(-1, 16)
    idxT[:, :] = seg.T.reshape(16, total // 16)
    idxT = np.tile(idxT, (8, 1))
    dstT = np.ascontiguousarray(
        dstflat.reshape(nblk, P).T.astype(ml_dtypes.bfloat16)
    )
    return idxT, dstT


def compute_caps(binned_counts):
    """binned_counts: [n_cores, n_groups, nch, tpg] -> blocks per bin
    (max over cores, ceil /128)."""
    mx = binned_counts.max(axis=0)
    return np.maximum(1, -(-mx // P)).astype(np.int64)


_cache = {}


def kernel(x, edge_index):
    TILES = int(os.environ.get("KERNEL_TILES", "104"))
    TPG = int(os.environ.get("KERNEL_TPG", "13"))
    CHUNK = 25000
    DESC_ROWS = int(os.environ.get("KERNEL_DESC_ROWS", "1"))
    NBUF_M = int(os.environ.get("KERNEL_NBUF_M", "4"))
    NQ = 4

    x = np.asarray(x, dtype=np.float32)
    edge_index = np.asarray(edge_index)
    src = edge_index[0].astype(np.int64)
    dst = edge_index[1].astype(np.int64)

    n_groups = TILES // TPG
    spans = make_chunks(N_NODES, CHUNK)
    nch = len(spans)

    node_slot = balance_nodes(src, dst, N_CORES, TILES, spans)
    eslot = node_slot[dst]
    ecore = eslot // (TILES * P)
    erel = eslot % (TILES * P)

    # per-core bin counts for caps
    cid = np.minimum(src // CHUNK, nch - 1)
    t = erel >> 7
    bk = ((ecore * n_groups + t // TPG) * nch + cid) * TPG + (t % TPG)
    bc = np.bincount(bk, minlength=N_CORES * n_groups * nch * TPG).reshape(
        N_CORES, n_groups, nch, TPG
    )
    caps = compute_caps(bc)

    key = (caps.tobytes(), n_groups, TPG, DESC_ROWS, NBUF_M)
    if key not in _cache:
        _cache[key] = build_program(
            spans, caps, n_groups, TPG, N_CORES, DESC_ROWS, NBUF_M, NQ
        )
    nc = _cache[key]

    xbf = np.zeros((N_NODES + 2 * DESC_ROWS, D), ml_dtypes.bfloat16)
    xbf[:N_NODES] = x.astype(ml_dtypes.bfloat16)
    iota = np.tile(
        np.arange(P, dtype=np.float32).astype(ml_dtypes.bfloat16), (P, 1)
    )
    in_maps = []
    for k in range(N_CORES):
        m = ecore == k
        idxT, dstT = prep_core(
            src[m], erel[m], spans, caps, n_groups, TPG, NBUF_M
        )
        in_maps.append({"xbf": xbf, "idxT": idxT, "dstT": dstT, "iota": iota})

    trace = bool(int(os.environ.get("KERNEL_TRACE", "0")))
    res = run_bass_kernel_spmd(
        nc, in_maps, core_ids=list(range(N_CORES)), trace=trace
    )
    if trace:
        kernel.last_results = res
    dev = np.stack([res.results[c]["out"] for c in range(N_CORES)])  # [8, T*128, D]
    full = np.empty((N_NODES, D), np.float32)
    full[:] = dev.reshape(N_CORES * TILES * P, D)[node_slot]
    return np.ascontiguousarray(full)


# revision 32
# speedup vs baseline: 1.3124x; 1.3124x over previous
"""GNN message passing (gather + scatter-add) on 8 trn2 NeuronCores, v5.

Strategy (dst-sharded, gather via InstDMAGatherAnt on 4 SWDGE queues):
  * The host ASSIGNS dst nodes to (core, tile, partition) slots with a
    load balancer (serpentine deal on degree + per-(tile,chunk) repair
    swaps, max bin 496 vs Poisson-max ~594) so every (group, chunk,
    tile) bin fits exactly 4 blocks of 128. 104 tiles per core, TPG=13
    (4 PSUM banks per group, 2 groups in flight), 416 bins of mean 481.
  * x stays in HBM as bf16. Edges are binned by (group, chunk=25000 src
    rows, tile), sorted by src within bins. Adjacent tile bins merge
    into one dma_gather of <=1024 ring descriptors (chunk-relative
    int16 idx). NO per-gather reg_load: every gather passes a shared
    capacity register (reg_mov once per distinct size); non-final bins
    pad with idx 0 (dst code 255 => sel column 0) and the final bin's
    trailing -1s are trimmed by the ucode, recovering per-core counts.
  * Pool-engine descriptor generation is THE bottleneck (~2.6 ns/desc
    marginal, 4-queue SWDGE; ~206k descriptors/core). SWDGE ring is
    2048 descs (dynamic_dma_scratch_size=32768) and single_packet=False,
    both measured slightly faster. First NBUF_M steps gather at full
    capacity so msg buffers need no memset (stale slots stay finite).
  * One DVE is_equal per (group, chunk) step builds one-hot sel planes;
    one matmul per 128-slot block accumulates psum[dst,f] += sel^T@msg.
    PSUM start/stop once per bank per group phase. psum -> SBUF via
    scalar engine (whole-bank copies), HWDGE DMA out; host un-permutes
    rows via the balancer's node map. No collective needed.

Measured: ~610 us median (best 596, run-to-run +/-4%), rel err 1.9e-3;
baseline v3 was 837 us. Span = ramp ~17 + Pool gather stream ~577
(206k descs x ~2.6ns + 224 gathers x ~0.3us fixed, gen serialized on
the Pool engine) + tail ~26. Verified dead ends ON HW this session:
>1024 descs/gather aborts regardless of ring size (Q7 scratch cap);
512B descriptors (desc_rows=2) win 16%/desc in microbench but lose
in-kernel (byte pressure); fp8 sel planes (mixed fp8xbf16 matmul is
exact but 25% slower end-to-end); per-gather idx-slice DMAs (224 small
HWDGE DMAs fight the SWDGE stream); Q7 compute gathers (ap_gather et
al) are slower per idx than SWDGE descgen and also Pool-serialized.
"""

import os
import sys

import numpy as np
import ml_dtypes

for _p in ("/opt/trn_rl_repo",):
    if _p not in sys.path:
        sys.path.insert(0, _p)

import bass_rust  # noqa: E402
from concourse import bass, mybir, tile, bacc, library_config  # noqa: E402
from concourse.bass_utils import run_bass_kernel_spmd  # noqa: E402

P = 128
D = 128
N_NODES = 100000
N_CORES = 8

NBUF_S = int(os.environ.get("KERNEL_NBUF_S", "3"))  # sel buffers


def make_chunks(n_src, chunk):
    spans = []
    b = 0
    while b < n_src:
        s = min(chunk, n_src - b)
        spans.append((b, s))
        b += s
    return spans


def balance_nodes(src, dst, n_cores, tiles, spans):
    """Assign each dst node a (core, tile, partition) slot, balancing the
    per-(tile, chunk) edge counts. Returns node_slot [N] int64 encoding
    core*tiles*128 + tile*128 + p, with every tile holding <=128 nodes."""
    n = N_NODES
    nch = len(spans)
    ntile = n_cores * tiles
    # per-node per-chunk degree
    cid = np.minimum(src // spans[0][1], nch - 1)
    degc = np.zeros((nch, n), np.int32)
    for c in range(nch):
        degc[c] = np.bincount(dst[cid == c], minlength=n)
    deg = degc.sum(axis=0)

    # serpentine deal on total degree: round r gives one node to each tile,
    # pairing heaviest remaining nodes with lightest tiles.
    order = np.argsort(-deg, kind="stable")
    loads = np.zeros(ntile, np.int64)
    fill = np.zeros(ntile, np.int32)
    assign = np.empty(n, np.int64)
    pos = 0
    while pos < n:
        batch = order[pos : pos + ntile]
        tl = np.argsort(loads, kind="stable")[: len(batch)]
        assign[batch] = tl
        loads[tl] += deg[batch]
        fill[tl] += 1
        pos += len(batch)

    # repair pass: per-(tile, chunk) loads; swap high-deg nodes out of
    # overloaded bins into the lightest tiles (matched by total degree).
    cl = np.zeros((ntile, nch), np.int64)
    for c in range(nch):
        np.add.at(cl[:, c], assign, degc[c])
    lim = int(cl.mean()) + 16
    for _ in range(3000):
        worst = np.unravel_index(np.argmax(cl), cl.shape)
        t0, c0 = int(worst[0]), int(worst[1])
        if cl[t0, c0] <= lim:
            break
        cand = np.flatnonzero(assign == t0)
        mover = cand[np.argmax(degc[c0, cand])]
        t1 = int(np.argmin(cl[:, c0] + (fill >= 128) * (1 << 40)))
        # swap mover with a node in t1 of similar total degree but low c0 deg
        cand1 = np.flatnonzero(assign == t1)
        recv = cand1[np.argmin(degc[c0, cand1].astype(np.int64) * (1 << 20) - deg[cand1])]
        if degc[c0, mover] <= degc[c0, recv]:
            break
        assign[mover], assign[recv] = t1, t0
        cl[t0] += degc[:, recv] - degc[:, mover]
        cl[t1] += degc[:, mover] - degc[:, recv]

    # partition index within tile
    order2 = np.argsort(assign, kind="stable")
    idx_in_tile = np.empty(n, np.int64)
    start = 0
    counts = np.bincount(assign, minlength=ntile)
    assert counts.max() <= 128
    off = np.concatenate([[0], np.cumsum(counts)])
    ranks = np.arange(n) - off[assign[order2]]
    idx_in_tile[order2] = ranks
    node_slot = assign * P + idx_in_tile
    return node_slot  # global slot id: (core*tiles + tile)*128 + p


def merge_plan(blocks_step, max_blocks=8):
    """Greedy merge of adjacent tile bins into gathers of <= max_blocks*128
    descriptors (the SWDGE ring cap). Returns [(t_start, t_end_incl), ...]."""
    plan = []
    t = 0
    tpg = len(blocks_step)
    while t < tpg:
        e = t
        acc = int(blocks_step[t])
        while e + 1 < tpg and acc + int(blocks_step[e + 1]) <= max_blocks:
            e += 1
            acc += int(blocks_step[e])
        plan.append((t, e))
        t = e + 1
    return plan


def build_program(spans, caps, n_groups, tpg, num_devices, desc_rows, nbuf_m, nq):
    """caps: int array [n_groups, nch, tpg] = blocks per bin (uniform across
    cores). Gathers merge adjacent tile bins (<=1024 ring descriptors each).
    Output rows: n_groups*tpg*128."""
    nch = len(spans)
    E = D * desc_rows  # gathered elems per slot
    blocks = np.asarray(caps)  # [g][c][t]
    nblk = int(blocks.sum())
    step_blocks = blocks.sum(axis=2)  # [g][c]
    max_nb = int(step_blocks.max())
    nbins = n_groups * nch * tpg

    nc = bacc.Bacc(
        "TRN2",
        target_bir_lowering=False,
        debug=False,
        num_devices=num_devices,
        num_swdge_queues=nq,
        # ring of 2048 descriptors per SWDGE queue: two 1024-desc gathers in
        # flight per queue (deeper gen/drain pipelining, measured ~15% faster
        # per descriptor than the default 1024-desc ring).
        dynamic_dma_scratch_size=32768,
    )
    n_src = spans[-1][0] + spans[-1][1]
    xbf = nc.dram_tensor(
        "xbf", [n_src + 2 * desc_rows, D], mybir.dt.bfloat16, kind="ExternalInput"
    ).ap()
    ncol = nblk * P // 16
    idxT = nc.dram_tensor("idxT", [P, ncol], mybir.dt.int16, kind="ExternalInput").ap()
    dstT = nc.dram_tensor(
        "dstT", [P, nblk], mybir.dt.bfloat16, kind="ExternalInput"
    ).ap()
    iota = nc.dram_tensor(
        "iota", [P, P], mybir.dt.bfloat16, kind="ExternalInput"
    ).ap()
    # output kept in the staging layout [group][partition][tile*D]: one
    # contiguous 850KB DMA per group; the host untangles rows for free
    out = nc.dram_tensor(
        "out", [n_groups, P, tpg * D], mybir.dt.float32, kind="ExternalOutput"
    ).ap()

    # slot offset (in blocks) of each bin, ordered (g, c, t)
    boff = np.zeros(nbins + 1, np.int64)
    boff[1:] = np.cumsum(blocks.reshape(-1))

    def bin_id(g, c, t):
        return (g * nch + c) * tpg + t

    with tile.TileContext(nc) as tc:
        with tc.tile_pool(name="sb", bufs=1) as pool, tc.tile_pool(
            name="ps", bufs=1, space="PSUM"
        ) as psp:
            idxs = pool.tile([P, ncol], mybir.dt.int16)
            dsts = pool.tile([P, nblk], mybir.dt.bfloat16)
            iot = pool.tile([P, P], mybir.dt.bfloat16)
            # per-step idx slices: the first gathers only wait on their own
            # slice, not the whole 4 MB index upload
            for g in range(n_groups):
                for c in range(nch):
                    s0 = int(boff[bin_id(g, c, 0)]) * P // 16
                    s1 = (
                        int(boff[bin_id(g, c, tpg - 1)] + blocks[g, c, tpg - 1])
                        * P
                        // 16
                    )
                    nc.sync.dma_start(out=idxs[:, s0:s1], in_=idxT[:, s0:s1])
            nc.sync.dma_start(out=dsts[:], in_=dstT[:])
            nc.sync.dma_start(out=iot[:], in_=iota[:])
            nc.gpsimd.load_library(library_config.mlp)

            msg = [
                pool.tile([P, max_nb, E], mybir.dt.bfloat16, name=f"msg{i}")
                for i in range(nbuf_m)
            ]
            sel = [
                pool.tile([P, max_nb, P], mybir.dt.bfloat16, name=f"sel{i}")
                for i in range(NBUF_S)
            ]
            stg = [
                pool.tile([P, tpg * D], mybir.dt.float32, name=f"stg{i}")
                for i in range(2)
            ]
            # no msg memsets: the first nbuf_m steps gather at FULL capacity
            # (host pads with idx 0 / dst 255), so stale slots always hold
            # finite bf16 data from a real row thereafter.
            bpg = -(-tpg // 4)  # banks per group
            assert 2 * bpg <= 8
            banks = [
                psp.tile([P, 4 * D], dtype=mybir.dt.float32, space="PSUM", name=f"psb{j}")
                for j in range(2 * bpg)
            ]

            def pregion(g, t):
                bk = banks[(g % 2) * bpg + t // 4]
                return bk[:, (t % 4) * D : (t % 4 + 1) * D]

            # one register per distinct gather capacity, set once: the ucode's
            # trailing -1 trim recovers each core's actual count, so no
            # per-gather reg_load is needed.
            plans = {
                (g, c): merge_plan(blocks[g, c])
                for g in range(n_groups)
                for c in range(nch)
            }
            capregs = {}
            for (g, c), plan in plans.items():
                for t0, t1 in plan:
                    ns = int(
                        boff[bin_id(g, c, t1)]
                        + blocks[g, c, t1]
                        - boff[bin_id(g, c, t0)]
                    ) * P
                    if ns not in capregs:
                        capregs[ns] = nc.gpsimd.alloc_register(f"cap{ns}")
            for ns, r in capregs.items():
                nc.gpsimd.reg_mov(r, ns)

            step = 0
            gq = 0
            for g in range(n_groups):
                for c in range(nch):
                    km = step % nbuf_m
                    ks = step % NBUF_S
                    mg, sl = msg[km], sel[ks]
                    base, span = spans[c]
                    nb = int(step_blocks[g, c])
                    sb0 = boff[bin_id(g, c, 0)]  # first block of this step
                    inap = xbf[base : base + span + 2 * desc_rows, :]
                    if desc_rows > 1:
                        # overlapping window view: row i -> elems [i*D, i*D+E)
                        inap = inap.copy()
                        inap.ap = bass_rust.VecI64Pair(
                            [(D, span + desc_rows), (1, E)]
                        )
                    for t0, t1 in plans[(g, c)]:
                        b0 = boff[bin_id(g, c, t0)]
                        bend = boff[bin_id(g, c, t1)] + blocks[g, c, t1]
                        nslot = int(bend - b0) * P
                        coloff = int(b0) * P // 16
                        nc.gpsimd.dma_gather(
                            mg[:, int(b0 - sb0) : int(bend - sb0), :],
                            inap,
                            idxs[:, coloff : coloff + nslot // 16],
                            nslot,
                            capregs[nslot],
                            E,
                            elem_step=D,
                            queue_num=gq % nq,
                            single_packet=bool(
                                int(os.environ.get("KERNEL_SP", "0"))
                            ),
                        )
                        gq += 1
                    nc.vector.tensor_tensor(
                        out=sl[:, :nb, :],
                        in0=dsts[:, int(sb0) : int(sb0 + nb)][:, :, None].to_broadcast(
                            [P, nb, P]
                        ),
                        in1=iot[:, None, :].to_broadcast([P, nb, P]),
                        op=mybir.AluOpType.is_equal,
                    )
                    # bank-interleaved tile order, block-outer: consecutive
                    # matmuls never hit the same psum region/bank.
                    torder = [t for r in range(4) for t in range(r, tpg, 4)]
                    last_of_bank = {}
                    maxb = int(blocks[g, c].max())
                    for b in range(maxb):
                        for t in torder:
                            if b < blocks[g, c, t]:
                                last_of_bank[t // 4] = (t, b)
                    started = set()
                    for b in range(maxb):
                        for t in torder:
                            if b >= blocks[g, c, t]:
                                continue
                            j = int(boff[bin_id(g, c, t)] - sb0) + b
                            bank = t // 4
                            start = c == 0 and b == 0 and bank not in started
                            if start:
                                started.add(bank)
                            nc.tensor.matmul(
                                out=pregion(g, t),
                                lhsT=sl[:, j, :],
                                rhs=mg[:, j, 0:D],
                                start=start,
                                stop=(
                                    c == nch - 1
                                    and last_of_bank[bank] == (t, b)
                                ),
                            )
                    step += 1
                sg = stg[g % 2]
                for k in range(bpg):
                    w = min(4, tpg - 4 * k)
                    bk = banks[(g % 2) * bpg + k]
                    nc.scalar.copy(
                        sg[:, 4 * k * D : (4 * k + w) * D], bk[:, : w * D]
                    )
                nc.sync.dma_start(out=out[g], in_=sg[:])

    for blk in nc.main_func.blocks:
        for ins in blk.instructions:
            if isinstance(ins, mybir.InstDMAGatherAnt):
                si = ins.sync_info
                if si and si.on_update:
                    name = si.on_update[0].ant_name
                    lane = int(name.split("_")[0][len("DMASW") :])
                    ins.queue_num = lane % nq
    nc.compile()
    return nc


def prep_core(src, rel, spans, caps, n_groups, tpg, nbuf_m=3):
    """Bin one core's edges (src global, rel = tile*128+p core-relative slot)
    into the (group, chunk, tile) layout. Returns (idxT, dstT).

    Gathers pass the (compile-time) capacity register; per-core counts are
    recovered by the ucode's trailing -1 trim. Mid-pads (first bin of each
    merged pair, and ALL pads in the first nbuf_m steps so msg buffers get
    fully initialized without memsets) are idx 0 with dst code 255."""
    nch = len(spans)
    blocks = np.asarray(caps)
    nblk = int(blocks.sum())
    nbins = n_groups * nch * tpg
    boff = np.zeros(nbins + 1, np.int64)
    boff[1:] = np.cumsum(blocks.reshape(-1))

    t = rel >> 7
    g = t // tpg
    ti = t % tpg
    chunk = spans[0][1]
    c = np.minimum(src // chunk, nch - 1)
    bases = np.array([b for b, s in spans], dtype=np.int64)
    sr = src - bases[c]

    binkey = (g * nch + c) * tpg + ti
    order = np.lexsort((sr, binkey))
    sr, relo, binkey = sr[order], rel[order], binkey[order]
    counts = np.bincount(binkey, minlength=nbins)
    capacity = blocks.reshape(-1) * P
    if (counts > capacity).any():
        raise ValueError("caps too small")
    starts = np.zeros(nbins, np.int64)
    starts[1:] = np.cumsum(counts)[:-1]
    pos = np.arange(len(sr)) - starts[binkey]
    slot = boff[binkey] * P + pos

    total = nblk * P
    srcflat = np.full(total, -1, np.int64)
    dstflat = np.full(total, 255, np.int64)  # 255 = stale/pad (sel col 0)
    srcflat[slot] = sr
    dstflat[slot] = relo & 127

    # pad everything except each merged gather's trailing bin with idx 0
    pad0 = np.zeros(nbins, bool)
    stepi = 0
    for gg in range(n_groups):
        for cc in range(nch):
            plan = merge_plan(blocks[gg, cc])
            for t0, t1 in plan:
                for tt in range(t0, t1 + 1):
                    k = (gg * nch + cc) * tpg + tt
                    pad0[k] = (tt < t1) or (stepi < nbuf_m)
            stepi += 1
    binf = np.repeat(np.arange(nbins), blocks.reshape(-1) * P)
    fill = (srcflat < 0) & pad0[binf]
    srcflat[fill] = 0

    idxT = np.empty((16, total // 16), np.int16)
    seg = srcflat.reshape(-1, 16)
    idxT[:, :] = seg.T.reshape(16, total // 16)
    idxT = np.tile(idxT, (8, 1))
    dstT = np.ascontiguousarray(
        dstflat.reshape(nblk, P).T.astype(ml_dtypes.bfloat16)
    )
    return idxT, dstT


def compute_caps(binned_counts):
    """binned_counts: [n_cores, n_groups, nch, tpg] -> blocks per bin
    (max over cores, ceil /128)."""
    mx = binned_counts.max(axis=0)
    return np.maximum(1, -(-mx // P)).astype(np.int64)


_cache = {}


def kernel(x, edge_index):
    TILES = int(os.environ.get("KERNEL_TILES", "104"))
    TPG = int(os.environ.get("KERNEL_TPG", "13"))
    CHUNK = 25000
    DESC_ROWS = int(os.environ.get("KERNEL_DESC_ROWS", "1"))
    NBUF_M = int(os.environ.get("KERNEL_NBUF_M", "4"))
    NQ = 4

    x = np.asarray(x, dtype=np.float32)
    edge_index = np.asarray(edge_index)
    src = edge_index[0].astype(np.int64)
    dst = edge_index[1].astype(np.int64)

    n_groups = TILES // TPG
    spans = make_chunks(N_NODES, CHUNK)
    nch = len(spans)

    node_slot = balance_nodes(src, dst, N_CORES, TILES, spans)
    eslot = node_slot[dst]
    ecore = eslot // (TILES * P)
    erel = eslot % (TILES * P)

    # per-core bin counts for caps
    cid = np.minimum(src // CHUNK, nch - 1)
    t = erel >> 7
    bk = ((ecore * n_groups + t // TPG) * nch + cid) * TPG + (t % TPG)
    bc = np.bincount(bk, minlength=N_CORES * n_groups * nch * TPG).reshape(
        N_CORES, n_groups, nch, TPG
    )
    caps = compute_caps(bc)

    key = (caps.tobytes(), n_groups, TPG, DESC_ROWS, NBUF_M)
    if key not in _cache:
        _cache[key] = build_program(
            spans, caps, n_groups, TPG, N_CORES, DESC_ROWS, NBUF_M, NQ
        )
    nc = _cache[key]

    xbf = np.zeros((N_NODES + 2 * DESC_ROWS, D), ml_dtypes.bfloat16)
    xbf[:N_NODES] = x.astype(ml_dtypes.bfloat16)
    iota = np.tile(
        np.arange(P, dtype=np.float32).astype(ml_dtypes.bfloat16), (P, 1)
    )
    in_maps = []
    for k in range(N_CORES):
        m = ecore == k
        idxT, dstT = prep_core(
            src[m], erel[m], spans, caps, n_groups, TPG, NBUF_M
        )
        in_maps.append({"xbf": xbf, "idxT": idxT, "dstT": dstT, "iota": iota})

    trace = bool(int(os.environ.get("KERNEL_TRACE", "0")))
    res = run_bass_kernel_spmd(
        nc, in_maps, core_ids=list(range(N_CORES)), trace=trace
    )
    if trace:
        kernel.last_results = res
    # device layout [n_groups, P, tpg*D] -> rows (g*tpg+t)*128+p
    dev = np.stack(
        [
            np.asarray(res.results[c]["out"])
            .reshape(n_groups, P, TPG, D)
            .transpose(0, 2, 1, 3)
            .reshape(TILES * P, D)
            for c in range(N_CORES)
        ]
    )
    full = np.empty((N_NODES, D), np.float32)
    full[:] = dev.reshape(N_CORES * TILES * P, D)[node_slot]
    return np.ascontiguousarray(full)
